# revision 5
# baseline (speedup 1.0000x reference)
"""GCN encoder (2-layer, mu/logstd heads) on 8 Trainium2 NeuronCores.

Math (exactly matches the reference up to fp rounding):
    deg  = indeg(dst) + 1 ; dinv = deg**-0.5
    norm_e = dinv[src]*dinv[dst]  (separable!)
    conv(h, W, b) = dinv * (segsum(h~[src]) + h~) @ W + b   with  h~ = dinv*h
    h1 = relu(conv(drop(x), W1, b1))
    (mu | logstd) = dinv * (segsum(h1~[src]) + h1~) @ [Wmu|Wls] + [bmu|bls]

Distribution: nodes are greedy-packed (by in-degree) into 8 cores x 49
row-tiles of <=128 nodes so every tile has ~equal edge load; each core owns
the aggregation for its tiles (scatter-adds local). Between the two passes
the per-core h1~ shards are AllGathered so pass-2 gathers are local.

Per tile: indirect-DMA gathers 128 source rows/instruction into SBUF; a
DVE-built one-hot (edges x local-dst) matmul on the PE does the
segment-sum into PSUM at full f32r rate; epilogue adds the self row,
scales by dinv, transposes via PE, and applies the weight matmul.
"""

import numpy as np

N_NODES = 50000
N_EDGES = 800000
D_IN = 512
D_H = 512
D_OUT = 256
N_CORES = 8
P = 128
TILES = 49  # per core; tiles 0..47 have 128 rows, tile 48 has 106
LAST_ROWS = N_NODES // N_CORES - 48 * P  # 106
PAD_ROWS = TILES * P  # 6272 padded rows per core in the table row space
LOC_ROWS = N_NODES // N_CORES  # 6250
GROUP = 4  # dst tiles per weight-matmul group

_COMPILED = {}  # kmax -> (nc, out_names)


def _build(kmax: int):
    from concourse import bacc, bass, mybir, tile
    from concourse.masks import make_identity

    f32 = mybir.dt.float32
    f32r = mybir.dt.float32r
    i32 = mybir.dt.int32

    nc = bacc.Bacc("TRN2", target_bir_lowering=False, debug=False,
                   num_devices=N_CORES)

    table = nc.dram_tensor("table", [N_CORES * PAD_ROWS, D_IN], f32r,
                           kind="ExternalInput").ap()
    idx_all = nc.dram_tensor("idx_all", [TILES, P, kmax], i32,
                             kind="ExternalInput").ap()
    dstloc_all = nc.dram_tensor("dstloc_all", [TILES, P, kmax], f32,
                                kind="ExternalInput").ap()
    dinv_nm = nc.dram_tensor("dinv_nm", [PAD_ROWS, 1], f32,
                             kind="ExternalInput").ap()
    dinvb = nc.dram_tensor("dinvb", [P, PAD_ROWS], f32,
                           kind="ExternalInput").ap()
    w1 = nc.dram_tensor("w1", [D_IN, D_H], f32r, kind="ExternalInput").ap()
    wcat = nc.dram_tensor("wcat", [D_H, 2 * D_OUT], f32r,
                          kind="ExternalInput").ap()
    b1t = nc.dram_tensor("b1t", [P, D_H // P], f32, kind="ExternalInput").ap()
    bcatt = nc.dram_tensor("bcatt", [P, (2 * D_OUT) // P], f32,
                           kind="ExternalInput").ap()
    iota_in = nc.dram_tensor("iota", [P, P], f32, kind="ExternalInput").ap()
    self_tab = nc.dram_tensor("self_tab", [PAD_ROWS, D_IN], f32r,
                              kind="ExternalInput").ap()
    out_cat = nc.dram_tensor("out_cat", [LOC_ROWS, 2 * D_OUT], f32,
                             kind="ExternalOutput").ap()

    h1_loc = nc.dram_tensor("h1_loc", [PAD_ROWS, D_H], f32r).ap()
    h1_full = nc.dram_tensor("h1_full", [N_CORES * PAD_ROWS, D_H], f32r,
                             addr_space="Shared").ap()

    groups = [list(range(g, min(g + GROUP, TILES)))
              for g in range(0, TILES, GROUP)]

    with tile.TileContext(nc) as tc:
        with (
            tc.tile_pool(name="const", bufs=1) as cpool,
            tc.tile_pool(name="sbuf", bufs=2) as sbuf,
            tc.tile_pool(name="gat", bufs=2) as gatp,
            tc.tile_pool(name="stage", bufs=2) as stg,
            tc.tile_pool(name="psum_agg", bufs=2, space="PSUM") as ps_agg,
            tc.tile_pool(name="psum_tr", bufs=2, space="PSUM") as ps_trp,
            tc.tile_pool(name="psum_mm", bufs=2, space="PSUM") as ps_mmp,
        ):
            ident = cpool.tile([P, P], f32)
            make_identity(nc, ident[:])
            iota_sb = cpool.tile([P, P], f32)
            nc.sync.dma_start(out=iota_sb[:], in_=iota_in[:, :])
            w1_sb = []
            wcat_sb = []
            for k in range(4):
                t = cpool.tile([P, D_H], f32r, tag=f"w1_{k}")
                nc.sync.dma_start(out=t[:], in_=w1[k * P:(k + 1) * P, :])
                w1_sb.append(t)
                t2 = cpool.tile([P, 2 * D_OUT], f32r, tag=f"wc_{k}")
                nc.sync.dma_start(out=t2[:], in_=wcat[k * P:(k + 1) * P, :])
                wcat_sb.append(t2)
            b1_sb = cpool.tile([P, D_H // P], f32)
            nc.sync.dma_start(out=b1_sb[:], in_=b1t[:, :])
            bcat_sb = cpool.tile([P, (2 * D_OUT) // P], f32)
            nc.sync.dma_start(out=bcat_sb[:], in_=bcatt[:, :])

            def run_pass(src_table, self_table, self_base_mul, weights, bias_sb,
                         relu, dinv_post, dst_dram, hnm_dt):
                d_out2 = weights[0].shape[1]
                jb = d_out2 // P
                for g_i, g_tiles in enumerate(groups):
                    gn = len(g_tiles)
                    bt = [stg.tile([P, GROUP * P], f32r, tag=f"bt{k}",
                                   name=f"bt{k}") for k in range(4)]
                    hnm = [stg.tile([P, d_out2], hnm_dt, tag=f"hnm{i}",
                                    name=f"hnm{i}") for i in range(GROUP)]
                    for ti, t in enumerate(g_tiles):
                        rows = P if t < TILES - 1 else LAST_ROWS
                        base = t * P
                        idx_sb = sbuf.tile([P, kmax], i32, tag="idx")
                        nc.sync.dma_start(out=idx_sb[:], in_=idx_all[t, :, :])
                        dl_sb = sbuf.tile([P, kmax], f32, tag="dl")
                        nc.sync.dma_start(out=dl_sb[:], in_=dstloc_all[t, :, :])
                        g_t = gatp.tile([P, kmax, D_IN], f32r, tag="G")
                        agg = ps_agg.tile([P, D_IN], f32, space="PSUM",
                                          tag="agg")
                        for c in range(kmax):
                            nc.gpsimd.indirect_dma_start(
                                out=g_t[:, c, :],
                                out_offset=None,
                                in_=src_table,
                                in_offset=bass.IndirectOffsetOnAxis(
                                    ap=idx_sb[:, c:c + 1], axis=0),
                            )
                            oh = sbuf.tile([P, P], f32r, tag="oh")
                            nc.vector.tensor_tensor(
                                out=oh[:],
                                in0=dl_sb[:, c:c + 1].to_broadcast([P, P]),
                                in1=iota_sb[:],
                                op=mybir.AluOpType.is_equal,
                            )
                            nc.tensor.matmul(
                                out=agg[:], lhsT=oh[:], rhs=g_t[:, c, :],
                                start=(c == 0), stop=(c == kmax - 1),
                            )
                        self_sb = sbuf.tile([P, D_IN], f32r, tag="self")
                        nc.sync.dma_start(
                            out=self_sb[:],
                            in_=self_table[self_base_mul * 0 + base:
                                           base + P, :])
                        dv_sb = sbuf.tile([P, 1], f32, tag="dv")
                        nc.sync.dma_start(out=dv_sb[:],
                                          in_=dinv_nm[base:base + P, :])
                        b_sb = sbuf.tile([P, D_IN], f32, tag="B")
                        nc.vector.tensor_tensor(
                            out=b_sb[:], in0=agg[:],
                            in1=self_sb[:].bitcast(f32),
                            op=mybir.AluOpType.add)
                        b2_sb = sbuf.tile([P, D_IN], f32, tag="B2")
                        nc.vector.tensor_tensor(
                            out=b2_sb[:], in0=b_sb[:],
                            in1=dv_sb[:].to_broadcast([P, D_IN]),
                            op=mybir.AluOpType.mult)
                        for k in range(4):
                            ptr = ps_trp.tile([P, P], f32, space="PSUM",
                                              tag="tr")
                            nc.tensor.transpose(
                                out=ptr[:], in_=b2_sb[:, k * P:(k + 1) * P],
                                identity=ident[:])
                            nc.vector.tensor_copy(
                                out=bt[k][:, ti * P:(ti + 1) * P], in_=ptr[:])
                    # weight matmul over the group
                    nw = gn * P
                    for j in range(jb):
                        pmm = ps_mmp.tile([P, GROUP * P], f32, space="PSUM",
                                          tag="mm")
                        for k in range(4):
                            nc.tensor.matmul(
                                out=pmm[:, :nw],
                                lhsT=weights[k][:, j * P:(j + 1) * P],
                                rhs=bt[k][:, :nw],
                                start=(k == 0), stop=(k == 3))
                        hjt = sbuf.tile([P, GROUP * P], f32, tag="hjt")
                        if relu:
                            nc.scalar.activation(
                                out=hjt[:, :nw], in_=pmm[:, :nw],
                                func=mybir.ActivationFunctionType.Relu,
                                bias=bias_sb[:, j:j + 1])
                        else:
                            nc.vector.tensor_tensor(
                                out=hjt[:, :nw], in0=pmm[:, :nw],
                                in1=bias_sb[:, j:j + 1].to_broadcast(
                                    [P, nw]),
                                op=mybir.AluOpType.add)
                        if dinv_post:
                            dvb_sb = sbuf.tile([P, GROUP * P], f32, tag="dvb")
                            nc.sync.dma_start(
                                out=dvb_sb[:, :nw],
                                in_=dinvb[:, g_tiles[0] * P:
                                          g_tiles[0] * P + nw])
                            hjt2 = sbuf.tile([P, GROUP * P], f32, tag="hjt2")
                            nc.vector.tensor_tensor(
                                out=hjt2[:, :nw], in0=hjt[:, :nw],
                                in1=dvb_sb[:, :nw],
                                op=mybir.AluOpType.mult)
                            hsrc = hjt2
                        else:
                            hsrc = hjt
                        for ti in range(gn):
                            ptr2 = ps_trp.tile([P, P], f32, space="PSUM",
                                               tag="tr")
                            nc.tensor.transpose(
                                out=ptr2[:],
                                in_=hsrc[:, ti * P:(ti + 1) * P],
                                identity=ident[:])
                            nc.vector.tensor_copy(
                                out=hnm[ti][:, j * P:(j + 1) * P],
                                in_=ptr2[:])
                    for ti, t in enumerate(g_tiles):
                        rows = P if t < TILES - 1 else LAST_ROWS
                        base = t * P
                        nc.sync.dma_start(
                            out=dst_dram[base:base + rows, :],
                            in_=hnm[ti][:rows, :])

            # pass 1: x~ -> h1~ (relu, post-scale by dinv for pass-2 source)
            run_pass(table[:], self_tab, 0, w1_sb, b1_sb, True, True,
                     h1_loc, f32r)
            # allgather h1~ shards
            nc.gpsimd.collective_compute(
                "AllGather",
                mybir.AluOpType.bypass,
                replica_groups=[list(range(N_CORES))],
                ins=[h1_loc[:, :]],
                outs=[h1_full[:, :]],
            )
            # pass 2: h1~ -> (mu|logstd)
            run_pass(h1_full[:], h1_loc, 0, wcat_sb, bcat_sb, False, False,
                     out_cat, f32)

    nc.compile()
    return nc


def _prep(x, edge_index, W1, b1, Wmu, bmu, Wls, bls):
    import heapq

    src = edge_index[0].astype(np.int64)
    dst = edge_index[1].astype(np.int64)
    deg = np.bincount(dst, minlength=N_NODES).astype(np.float64) + 1.0
    dinv = (1.0 / np.sqrt(deg)).astype(np.float32)

    # dropout (must exactly match reference's jax key(42) mask)
    import jax
    with jax.default_device(jax.devices("cpu")[0]):
        keep = np.asarray(jax.random.bernoulli(
            jax.random.key(42), 0.5, (N_NODES, D_IN)))
    h = np.where(keep, x * np.float32(2.0), np.float32(0.0)).astype(np.float32)
    ht = h * dinv[:, None]

    # greedy-balance nodes into 392 tiles by in-degree
    degin = np.bincount(dst, minlength=N_NODES)
    order = np.argsort(-degin, kind="stable")
    ntile = N_CORES * TILES
    cap = np.full(ntile, P, dtype=np.int64)
    cap[TILES - 1::TILES] = LAST_ROWS
    heap = [(0, t) for t in range(ntile)]
    heapq.heapify(heap)
    rows_used = np.zeros(ntile, dtype=np.int64)
    tile_of = np.empty(N_NODES, dtype=np.int64)
    row_of = np.empty(N_NODES, dtype=np.int64)
    for n in order:
        while True:
            load, t = heapq.heappop(heap)
            if rows_used[t] < cap[t]:
                break
        tile_of[n] = t
        row_of[n] = rows_used[t]
        rows_used[t] += 1
        if rows_used[t] < cap[t]:
            heapq.heappush(heap, (load + degin[n], t))
    core_of = tile_of // TILES
    ltile_of = tile_of % TILES
    grow_of = core_of * PAD_ROWS + ltile_of * P + row_of  # padded table row
    lrow_of = ltile_of * P + row_of  # row within core (valid < 6250)

    # permuted gather table [N_CORES*PAD_ROWS, D_IN]
    table = np.zeros((N_CORES * PAD_ROWS, D_IN), dtype=np.float32)
    table[grow_of] = ht

    # per-core edge arrays
    e_tile = tile_of[dst]
    e_order = np.argsort(e_tile, kind="stable")
    e_tile_s = e_tile[e_order]
    src_s = grow_of[src[e_order]]
    dloc_s = row_of[dst[e_order]]
    counts = np.bincount(e_tile_s, minlength=ntile)
    kmax = int(np.ceil(counts.max() / P))
    starts = np.concatenate([[0], np.cumsum(counts)[:-1]])
    pos = np.arange(len(e_order)) - starts[e_tile_s]
    pp = pos % P
    cc = pos // P
    idx_all = np.zeros((N_CORES, TILES, P, kmax), dtype=np.int32)
    dstloc_all = np.full((N_CORES, TILES, P, kmax), -1.0, dtype=np.float32)
    e_core = e_tile_s // TILES
    e_lt = e_tile_s % TILES
    idx_all[e_core, e_lt, pp, cc] = src_s.astype(np.int32)
    dstloc_all[e_core, e_lt, pp, cc] = dloc_s.astype(np.float32)

    dinv_nm = np.ones((N_CORES, PAD_ROWS, 1), dtype=np.float32)
    dinv_nm[core_of, lrow_of, 0] = dinv
    dinvb = np.ascontiguousarray(
        np.broadcast_to(dinv_nm[:, None, :, 0], (N_CORES, P, PAD_ROWS)))

    wcat = np.concatenate([Wmu, Wls], axis=1).astype(np.float32)
    b1t = np.ascontiguousarray(b1.reshape(D_H // P, P).T.astype(np.float32))
    bcat = np.concatenate([bmu, bls]).astype(np.float32)
    bcatt = np.ascontiguousarray(
        bcat.reshape((2 * D_OUT) // P, P).T.astype(np.float32))
    iota = np.broadcast_to(np.arange(P, dtype=np.float32)[None, :],
                           (P, P)).copy()

    in_maps = []
    for c in range(N_CORES):
        in_maps.append({
            "table": table,
            "self_tab": table[c * PAD_ROWS:(c + 1) * PAD_ROWS],
            "idx_all": idx_all[c],
            "dstloc_all": dstloc_all[c],
            "dinv_nm": dinv_nm[c],
            "dinvb": dinvb[c],
            "w1": W1.astype(np.float32),
            "wcat": wcat,
            "b1t": b1t,
            "bcatt": bcatt,
            "iota": iota,
        })
    return in_maps, kmax, core_of, lrow_of


def run(inputs, trace=False):
    from concourse.bass_utils import run_bass_kernel_spmd

    in_maps, kmax, core_of, lrow_of = _prep(**inputs)
    if kmax not in _COMPILED:
        _COMPILED[kmax] = _build(kmax)
    nc = _COMPILED[kmax]
    res = run_bass_kernel_spmd(nc, in_maps, core_ids=list(range(N_CORES)),
                               trace=trace)
    outs = np.stack([res.results[c]["out_cat"] for c in range(N_CORES)])
    full = outs[core_of, lrow_of]  # [N_NODES, 512] in node order
    mu = np.ascontiguousarray(full[:, :D_OUT])
    logstd = np.ascontiguousarray(full[:, D_OUT:])
    return (mu, logstd), res


def kernel(**inputs):
    (mu, logstd), _ = run(inputs, trace=False)
    return (mu, logstd)


# revision 6
# speedup vs baseline: 1.0199x; 1.0199x over previous
"""GCN encoder (2-layer, mu/logstd heads) on 8 Trainium2 NeuronCores.

Math (exactly matches the reference up to fp rounding):
    deg  = indeg(dst) + 1 ; dinv = deg**-0.5
    norm_e = dinv[src]*dinv[dst]  (separable!)
    conv(h, W, b) = dinv * (segsum(h~[src]) + h~) @ W + b   with  h~ = dinv*h
    h1 = relu(conv(drop(x), W1, b1))
    (mu | logstd) = dinv * (segsum(h1~[src]) + h1~) @ [Wmu|Wls] + [bmu|bls]

Distribution: nodes are greedy-packed (by in-degree) into 8 cores x 49
row-tiles of <=128 nodes so every tile has ~equal edge load; each core owns
the aggregation for its tiles (scatter-adds local). Between the two passes
the per-core h1~ shards are AllGathered so pass-2 gathers are local.

Per tile: indirect-DMA gathers 128 source rows/instruction into SBUF; a
DVE-built one-hot (edges x local-dst) matmul on the PE does the
segment-sum into PSUM at full f32r rate; epilogue adds the self row,
scales by dinv, transposes via PE, and applies the weight matmul.
"""

import numpy as np

N_NODES = 50000
N_EDGES = 800000
D_IN = 512
D_H = 512
D_OUT = 256
N_CORES = 8
P = 128
TILES = 49  # per core; tiles 0..47 have 128 rows, tile 48 has 106
LAST_ROWS = N_NODES // N_CORES - 48 * P  # 106
PAD_ROWS = TILES * P  # 6272 padded rows per core in the table row space
LOC_ROWS = N_NODES // N_CORES  # 6250
GROUP = 4  # dst tiles per weight-matmul group

_COMPILED = {}  # kmax -> compiled Bacc program
_KEEP_MASK = None  # cached dropout mask (input-independent: key(42), fixed shape)


def _build(kmax: int):
    from concourse import bacc, bass, mybir, tile
    from concourse.masks import make_identity

    f32 = mybir.dt.float32
    f32r = mybir.dt.float32r
    i32 = mybir.dt.int32

    nc = bacc.Bacc("TRN2", target_bir_lowering=False, debug=False,
                   num_devices=N_CORES)

    table = nc.dram_tensor("table", [N_CORES * PAD_ROWS, D_IN], f32r,
                           kind="ExternalInput").ap()
    idx_all = nc.dram_tensor("idx_all", [TILES, P, kmax], i32,
                             kind="ExternalInput").ap()
    dstloc_all = nc.dram_tensor("dstloc_all", [TILES, P, kmax], f32,
                                kind="ExternalInput").ap()
    dinv_nm = nc.dram_tensor("dinv_nm", [PAD_ROWS, 1], f32,
                             kind="ExternalInput").ap()
    dinvb = nc.dram_tensor("dinvb", [P, PAD_ROWS], f32,
                           kind="ExternalInput").ap()
    w1 = nc.dram_tensor("w1", [D_IN, D_H], f32r, kind="ExternalInput").ap()
    wcat = nc.dram_tensor("wcat", [D_H, 2 * D_OUT], f32r,
                          kind="ExternalInput").ap()
    b1t = nc.dram_tensor("b1t", [P, D_H // P], f32, kind="ExternalInput").ap()
    bcatt = nc.dram_tensor("bcatt", [P, (2 * D_OUT) // P], f32,
                           kind="ExternalInput").ap()
    iota_in = nc.dram_tensor("iota", [P, P], f32, kind="ExternalInput").ap()
    self_tab = nc.dram_tensor("self_tab", [PAD_ROWS, D_IN], f32r,
                              kind="ExternalInput").ap()
    out_cat = nc.dram_tensor("out_cat", [LOC_ROWS, 2 * D_OUT], f32,
                             kind="ExternalOutput").ap()

    h1_loc = nc.dram_tensor("h1_loc", [PAD_ROWS, D_H], f32r).ap()
    h1_full = nc.dram_tensor("h1_full", [N_CORES * PAD_ROWS, D_H], f32r,
                             addr_space="Shared").ap()

    groups = [list(range(g, min(g + GROUP, TILES)))
              for g in range(0, TILES, GROUP)]

    with tile.TileContext(nc) as tc:
        with (
            tc.tile_pool(name="const", bufs=1) as cpool,
            tc.tile_pool(name="sbuf", bufs=2) as sbuf,
            tc.tile_pool(name="gat", bufs=2) as gatp,
            tc.tile_pool(name="stage", bufs=2) as stg,
            tc.tile_pool(name="psum_agg", bufs=2, space="PSUM") as ps_agg,
            tc.tile_pool(name="psum_tr", bufs=2, space="PSUM") as ps_trp,
            tc.tile_pool(name="psum_mm", bufs=2, space="PSUM") as ps_mmp,
        ):
            ident = cpool.tile([P, P], f32)
            make_identity(nc, ident[:])
            iota_sb = cpool.tile([P, P], f32)
            nc.sync.dma_start(out=iota_sb[:], in_=iota_in[:, :])
            w1_sb = []
            wcat_sb = []
            for k in range(4):
                t = cpool.tile([P, D_H], f32r, tag=f"w1_{k}")
                nc.sync.dma_start(out=t[:], in_=w1[k * P:(k + 1) * P, :])
                w1_sb.append(t)
                t2 = cpool.tile([P, 2 * D_OUT], f32r, tag=f"wc_{k}")
                nc.sync.dma_start(out=t2[:], in_=wcat[k * P:(k + 1) * P, :])
                wcat_sb.append(t2)
            b1_sb = cpool.tile([P, D_H // P], f32)
            nc.sync.dma_start(out=b1_sb[:], in_=b1t[:, :])
            bcat_sb = cpool.tile([P, (2 * D_OUT) // P], f32)
            nc.sync.dma_start(out=bcat_sb[:], in_=bcatt[:, :])

            def run_pass(src_table, self_table, self_base_mul, weights, bias_sb,
                         relu, dinv_post, dst_dram, hnm_dt):
                d_out2 = weights[0].shape[1]
                jb = d_out2 // P
                for g_i, g_tiles in enumerate(groups):
                    gn = len(g_tiles)
                    bt = [stg.tile([P, GROUP * P], f32r, tag=f"bt{k}",
                                   name=f"bt{k}") for k in range(4)]
                    hnm = [stg.tile([P, d_out2], hnm_dt, tag=f"hnm{i}",
                                    name=f"hnm{i}") for i in range(GROUP)]
                    for ti, t in enumerate(g_tiles):
                        rows = P if t < TILES - 1 else LAST_ROWS
                        base = t * P
                        idx_sb = sbuf.tile([P, kmax], i32, tag="idx")
                        nc.sync.dma_start(out=idx_sb[:], in_=idx_all[t, :, :])
                        dl_sb = sbuf.tile([P, kmax], f32, tag="dl")
                        nc.sync.dma_start(out=dl_sb[:], in_=dstloc_all[t, :, :])
                        g_t = gatp.tile([P, kmax, D_IN], f32r, tag="G")
                        agg = ps_agg.tile([P, D_IN], f32, space="PSUM",
                                          tag="agg")
                        for c in range(kmax):
                            nc.gpsimd.indirect_dma_start(
                                out=g_t[:, c, :],
                                out_offset=None,
                                in_=src_table,
                                in_offset=bass.IndirectOffsetOnAxis(
                                    ap=idx_sb[:, c:c + 1], axis=0),
                            )
                            oh = sbuf.tile([P, P], f32r, tag="oh")
                            nc.vector.tensor_tensor(
                                out=oh[:],
                                in0=dl_sb[:, c:c + 1].to_broadcast([P, P]),
                                in1=iota_sb[:],
                                op=mybir.AluOpType.is_equal,
                            )
                            nc.tensor.matmul(
                                out=agg[:], lhsT=oh[:], rhs=g_t[:, c, :],
                                start=(c == 0), stop=(c == kmax - 1),
                            )
                        self_sb = sbuf.tile([P, D_IN], f32r, tag="self")
                        nc.sync.dma_start(
                            out=self_sb[:],
                            in_=self_table[self_base_mul * 0 + base:
                                           base + P, :])
                        dv_sb = sbuf.tile([P, 1], f32, tag="dv")
                        nc.sync.dma_start(out=dv_sb[:],
                                          in_=dinv_nm[base:base + P, :])
                        b_sb = sbuf.tile([P, D_IN], f32, tag="B")
                        nc.vector.tensor_tensor(
                            out=b_sb[:], in0=agg[:],
                            in1=self_sb[:].bitcast(f32),
                            op=mybir.AluOpType.add)
                        b2_sb = sbuf.tile([P, D_IN], f32, tag="B2")
                        nc.vector.tensor_tensor(
                            out=b2_sb[:], in0=b_sb[:],
                            in1=dv_sb[:].to_broadcast([P, D_IN]),
                            op=mybir.AluOpType.mult)
                        for k in range(4):
                            ptr = ps_trp.tile([P, P], f32, space="PSUM",
                                              tag="tr")
                            nc.tensor.transpose(
                                out=ptr[:], in_=b2_sb[:, k * P:(k + 1) * P],
                                identity=ident[:])
                            nc.vector.tensor_copy(
                                out=bt[k][:, ti * P:(ti + 1) * P], in_=ptr[:])
                    # weight matmul over the group
                    nw = gn * P
                    for j in range(jb):
                        pmm = ps_mmp.tile([P, GROUP * P], f32, space="PSUM",
                                          tag="mm")
                        for k in range(4):
                            nc.tensor.matmul(
                                out=pmm[:, :nw],
                                lhsT=weights[k][:, j * P:(j + 1) * P],
                                rhs=bt[k][:, :nw],
                                start=(k == 0), stop=(k == 3))
                        hjt = sbuf.tile([P, GROUP * P], f32, tag="hjt")
                        if relu:
                            nc.scalar.activation(
                                out=hjt[:, :nw], in_=pmm[:, :nw],
                                func=mybir.ActivationFunctionType.Relu,
                                bias=bias_sb[:, j:j + 1])
                        else:
                            nc.vector.tensor_tensor(
                                out=hjt[:, :nw], in0=pmm[:, :nw],
                                in1=bias_sb[:, j:j + 1].to_broadcast(
                                    [P, nw]),
                                op=mybir.AluOpType.add)
                        if dinv_post:
                            dvb_sb = sbuf.tile([P, GROUP * P], f32, tag="dvb")
                            nc.sync.dma_start(
                                out=dvb_sb[:, :nw],
                                in_=dinvb[:, g_tiles[0] * P:
                                          g_tiles[0] * P + nw])
                            hjt2 = sbuf.tile([P, GROUP * P], f32, tag="hjt2")
                            nc.vector.tensor_tensor(
                                out=hjt2[:, :nw], in0=hjt[:, :nw],
                                in1=dvb_sb[:, :nw],
                                op=mybir.AluOpType.mult)
                            hsrc = hjt2
                        else:
                            hsrc = hjt
                        for ti in range(gn):
                            ptr2 = ps_trp.tile([P, P], f32, space="PSUM",
                                               tag="tr")
                            nc.tensor.transpose(
                                out=ptr2[:],
                                in_=hsrc[:, ti * P:(ti + 1) * P],
                                identity=ident[:])
                            nc.vector.tensor_copy(
                                out=hnm[ti][:, j * P:(j + 1) * P],
                                in_=ptr2[:])
                    for ti, t in enumerate(g_tiles):
                        rows = P if t < TILES - 1 else LAST_ROWS
                        base = t * P
                        nc.sync.dma_start(
                            out=dst_dram[base:base + rows, :],
                            in_=hnm[ti][:rows, :])

            # pass 1: x~ -> h1~ (relu, post-scale by dinv for pass-2 source)
            run_pass(table[:], self_tab, 0, w1_sb, b1_sb, True, True,
                     h1_loc, f32r)
            # allgather h1~ shards
            nc.gpsimd.collective_compute(
                "AllGather",
                mybir.AluOpType.bypass,
                replica_groups=[list(range(N_CORES))],
                ins=[h1_loc[:, :]],
                outs=[h1_full[:, :]],
            )
            # pass 2: h1~ -> (mu|logstd)
            run_pass(h1_full[:], h1_loc, 0, wcat_sb, bcat_sb, False, False,
                     out_cat, f32)

    nc.compile()
    return nc


def _prep(x, edge_index, W1, b1, Wmu, bmu, Wls, bls):
    import heapq

    src = edge_index[0].astype(np.int64)
    dst = edge_index[1].astype(np.int64)
    deg = np.bincount(dst, minlength=N_NODES).astype(np.float64) + 1.0
    dinv = (1.0 / np.sqrt(deg)).astype(np.float32)

    # dropout (must exactly match reference's jax key(42) mask)
    global _KEEP_MASK
    if _KEEP_MASK is None:
        import jax
        with jax.default_device(jax.devices("cpu")[0]):
            _KEEP_MASK = np.asarray(jax.random.bernoulli(
                jax.random.key(42), 0.5, (N_NODES, D_IN)))
    keep = _KEEP_MASK
    h = np.where(keep, x * np.float32(2.0), np.float32(0.0)).astype(np.float32)
    ht = h * dinv[:, None]

    # greedy-balance nodes into 392 tiles by in-degree
    degin = np.bincount(dst, minlength=N_NODES)
    order = np.argsort(-degin, kind="stable")
    ntile = N_CORES * TILES
    cap = np.full(ntile, P, dtype=np.int64)
    cap[TILES - 1::TILES] = LAST_ROWS
    heap = [(0, t) for t in range(ntile)]
    heapq.heapify(heap)
    rows_used = np.zeros(ntile, dtype=np.int64)
    tile_of = np.empty(N_NODES, dtype=np.int64)
    row_of = np.empty(N_NODES, dtype=np.int64)
    for n in order:
        while True:
            load, t = heapq.heappop(heap)
            if rows_used[t] < cap[t]:
                break
        tile_of[n] = t
        row_of[n] = rows_used[t]
        rows_used[t] += 1
        if rows_used[t] < cap[t]:
            heapq.heappush(heap, (load + degin[n], t))
    core_of = tile_of // TILES
    ltile_of = tile_of % TILES
    grow_of = core_of * PAD_ROWS + ltile_of * P + row_of  # padded table row
    lrow_of = ltile_of * P + row_of  # row within core (valid < 6250)

    # permuted gather table [N_CORES*PAD_ROWS, D_IN]
    table = np.zeros((N_CORES * PAD_ROWS, D_IN), dtype=np.float32)
    table[grow_of] = ht

    # per-core edge arrays
    e_tile = tile_of[dst]
    e_order = np.argsort(e_tile, kind="stable")
    e_tile_s = e_tile[e_order]
    src_s = grow_of[src[e_order]]
    dloc_s = row_of[dst[e_order]]
    counts = np.bincount(e_tile_s, minlength=ntile)
    kmax = int(np.ceil(counts.max() / P))
    starts = np.concatenate([[0], np.cumsum(counts)[:-1]])
    pos = np.arange(len(e_order)) - starts[e_tile_s]
    pp = pos % P
    cc = pos // P
    idx_all = np.zeros((N_CORES, TILES, P, kmax), dtype=np.int32)
    dstloc_all = np.full((N_CORES, TILES, P, kmax), -1.0, dtype=np.float32)
    e_core = e_tile_s // TILES
    e_lt = e_tile_s % TILES
    idx_all[e_core, e_lt, pp, cc] = src_s.astype(np.int32)
    dstloc_all[e_core, e_lt, pp, cc] = dloc_s.astype(np.float32)

    dinv_nm = np.ones((N_CORES, PAD_ROWS, 1), dtype=np.float32)
    dinv_nm[core_of, lrow_of, 0] = dinv
    dinvb = np.ascontiguousarray(
        np.broadcast_to(dinv_nm[:, None, :, 0], (N_CORES, P, PAD_ROWS)))

    wcat = np.concatenate([Wmu, Wls], axis=1).astype(np.float32)
    b1t = np.ascontiguousarray(b1.reshape(D_H // P, P).T.astype(np.float32))
    bcat = np.concatenate([bmu, bls]).astype(np.float32)
    bcatt = np.ascontiguousarray(
        bcat.reshape((2 * D_OUT) // P, P).T.astype(np.float32))
    iota = np.broadcast_to(np.arange(P, dtype=np.float32)[None, :],
                           (P, P)).copy()

    in_maps = []
    for c in range(N_CORES):
        in_maps.append({
            "table": table,
            "self_tab": table[c * PAD_ROWS:(c + 1) * PAD_ROWS],
            "idx_all": idx_all[c],
            "dstloc_all": dstloc_all[c],
            "dinv_nm": dinv_nm[c],
            "dinvb": dinvb[c],
            "w1": W1.astype(np.float32),
            "wcat": wcat,
            "b1t": b1t,
            "bcatt": bcatt,
            "iota": iota,
        })
    return in_maps, kmax, core_of, lrow_of


def run(inputs, trace=False):
    from concourse.bass_utils import run_bass_kernel_spmd

    in_maps, kmax, core_of, lrow_of = _prep(**inputs)
    if kmax not in _COMPILED:
        _COMPILED[kmax] = _build(kmax)
    nc = _COMPILED[kmax]
    res = run_bass_kernel_spmd(nc, in_maps, core_ids=list(range(N_CORES)),
                               trace=trace)
    outs = np.stack([res.results[c]["out_cat"] for c in range(N_CORES)])
    full = outs[core_of, lrow_of]  # [N_NODES, 512] in node order
    mu = np.ascontiguousarray(full[:, :D_OUT])
    logstd = np.ascontiguousarray(full[:, D_OUT:])
    return (mu, logstd), res


def kernel(**inputs):
    (mu, logstd), _ = run(inputs, trace=False)
    return (mu, logstd)
